# revision 7
# baseline (speedup 1.0000x reference)
"""Contrastive-learning loss on 8 Trainium2 NeuronCores.

Math (see reference):
  sim = (X @ X.T) / 0.1                                   [E, E]
  M   = (|s_i - s_j| < 0.1) OR (edges i,j share endpoint, i != j)
  pos_i = sum_j M * sim ;  neg_i = sum_j exp(sim) * (1 - M)
  loss  = mean(-log(pos / (pos + neg + 1e-8)))

Strategy: sort edges by structural score. Then the score mask is a
contiguous band [lo_i, hi_i] per row, and the shared-endpoint mask is a
sparse O(E) correction (host). Device (per core, 1024 rows x 8192 cols):
  sim tile matmul (f32r) -> exp via ACT (scale=10 fused) with per-2048-block
  row-sum accumulation -> band-boundary partial sums via one fused
  DVE scalar_tensor_tensor (is_ge mask * exp, accumulated) per boundary
  block. pos comes from prefix sums of sorted embeddings (host, exact).

SPMD trick: one program runs on all 8 cores, so per-core inputs are
column-ROLLED so each core's row slab sits at local columns [2048, 3072),
making the boundary-block structure identical across cores.
"""
import sys

for _p in ("/opt/trn_rl_repo", "/root/.axon_site/_ro/trn_rl_repo"):
    if _p not in sys.path:
        sys.path.insert(0, _p)

import numpy as np

E = 8192
D = 256
NCORES = 8
SLAB = E // NCORES          # 1024 rows per core
SHIFT = 2048                # local column where the slab starts
NT = SLAB // 128            # 8 row-tiles per core
GRP = 2048                  # ACT/exp column group width
NG = E // GRP               # 4 groups
BLK = 512                   # boundary sub-block width
MAXC = 8                    # max boundary candidate blocks per side
TCOLS = 24                  # threshold-table columns per row-tile

_prog_cache: dict = {}
_last_run: dict = {}


def _round_fp32r(a: np.ndarray) -> np.ndarray:
    """Round fp32 to the PE's FP32R format (1-8-11, low 12 bits zero)."""
    b = np.ascontiguousarray(a, dtype=np.float32).view(np.uint32).copy()
    low = b & np.uint32(0xFFF)
    hi = b & ~np.uint32(0xFFF)
    up = (low > 0x800) | ((low == 0x800) & (((hi >> np.uint32(12)) & 1) == 1))
    hi = hi + (up.astype(np.uint32) << np.uint32(12))
    return hi.view(np.float32)


def _band_bounds(s: np.ndarray):
    """Per sorted row i: [lo_i, hi_i] = columns j with fp32 |s_i-s_j| < 0.1."""
    t = np.float32(0.1)
    n = len(s)
    idx = np.arange(n)
    lo = np.searchsorted(s, (s - t).astype(np.float32), side="left")
    # exact fp32-predicate fixups (predicate: s_i - s_j < 0.1 for j <= i)
    while True:
        m = (lo > 0) & ((s[idx] - s[np.maximum(lo - 1, 0)]).astype(np.float32) < t)
        if not m.any():
            break
        lo[m] -= 1
    while True:
        m = (lo < idx) & ~((s[idx] - s[lo]).astype(np.float32) < t)
        if not m.any():
            break
        lo[m] += 1
    hi = np.searchsorted(s, (s + t).astype(np.float32), side="right") - 1
    while True:
        m = (hi < n - 1) & ((s[np.minimum(hi + 1, n - 1)] - s[idx]).astype(np.float32) < t)
        if not m.any():
            break
        hi[m] += 1
    while True:
        m = (hi > idx) & ~((s[hi] - s[idx]).astype(np.float32) < t)
        if not m.any():
            break
        hi[m] -= 1
    assert (lo <= idx).all() and (hi >= idx).all()
    return lo.astype(np.int64), hi.astype(np.int64)


def _shared_pairs(edge_index: np.ndarray):
    """Unordered pairs (a, b), a<b, of edges sharing >=1 endpoint."""
    row = np.asarray(edge_index[0]).astype(np.int64)
    col = np.asarray(edge_index[1]).astype(np.int64)
    from collections import defaultdict

    node2edges = defaultdict(list)
    for e in range(len(row)):
        node2edges[int(row[e])].append(e)
        node2edges[int(col[e])].append(e)
    pairs = set()
    for lst in node2edges.values():
        if len(lst) < 2:
            continue
        uniq = sorted(set(lst))
        for i in range(len(uniq)):
            for j in range(i + 1, len(uniq)):
                pairs.add((uniq[i], uniq[j]))
    return pairs


def _build_structure(lo: np.ndarray, hi1: np.ndarray):
    """Static per-row-tile boundary-block structure, shared by all cores.

    lo / hi1 are in LOCAL (rolled) column coordinates, shape [NCORES, SLAB].
    Returns per-tile dicts with candidate 512-block ranges and the first
    full 2048-group index of the suffix, for each side.
    """
    struct = []
    for t in range(NT):
        rows = slice(t * 128, (t + 1) * 128)
        entry = {}
        for side, vals in (("lo", lo[:, rows]), ("hi", hi1[:, rows])):
            mn = int(vals.min())
            mx = int(vals.max())
            b0 = max(mn // BLK, 0)
            bmax = min(mx, E - 1) // BLK
            a = bmax // (GRP // BLK) + 1      # first full 2048-group of suffix
            b1 = a * (GRP // BLK) - 1          # last candidate 512-block
            C = b1 - b0 + 1
            assert C <= MAXC, f"boundary span too wide: tile {t} {side} C={C}"
            entry[side] = (b0, C, a)
        struct.append(entry)
    return struct


def _build_program(struct):
    import concourse.bass as bass
    import concourse.tile as tile
    from concourse import bacc, mybir

    f32 = mybir.dt.float32
    f32r = mybir.dt.float32r
    AF = mybir.ActivationFunctionType
    OP = mybir.AluOpType

    nc = bacc.Bacc("TRN2", target_bir_lowering=False, debug=False,
                   num_devices=NCORES)

    xa_d = nc.dram_tensor("xa", [2, 128, E], f32r, kind="ExternalInput")
    thr_d = nc.dram_tensor("thr", [128, NT * TCOLS], f32, kind="ExternalInput")
    iota_d = nc.dram_tensor("iota", [128, BLK], f32, kind="ExternalInput")
    s1_d = nc.dram_tensor("s1o", [128, NT], f32, kind="ExternalOutput")
    s3_d = nc.dram_tensor("s3o", [128, NT], f32, kind="ExternalOutput")

    with tile.TileContext(nc) as tc:
        with (
            tc.tile_pool(name="xin", bufs=1) as xin,
            tc.tile_pool(name="cst", bufs=1) as cst,
            tc.tile_pool(name="ps", bufs=2, space="PSUM") as psp,
            tc.tile_pool(name="expp", bufs=3) as expp,
            tc.tile_pool(name="accs", bufs=1) as accs,
            tc.tile_pool(name="junk", bufs=2) as junkp,
            tc.tile_pool(name="tiny", bufs=4) as tiny,
        ):
            # column-group chunked loads of the rolled, transposed embeddings
            xg = []      # xg[k][g]: [128, GRP] f32r
            for k in range(2):
                row = []
                for g in range(NG):
                    tl = xin.tile([128, GRP], f32r, tag=f"xa{k}_{g}")
                    nc.sync.dma_start(tl[:], xa_d.ap()[k, :, g * GRP:(g + 1) * GRP])
                    row.append(tl)
                xg.append(row)
            # slab (stationary operands) columns [SHIFT, SHIFT+SLAB)
            xs = []
            for k in range(2):
                tl = xin.tile([128, SLAB], f32r, tag=f"xs{k}")
                nc.sync.dma_start(tl[:], xa_d.ap()[k, :, SHIFT:SHIFT + SLAB])
                xs.append(tl)

            thr = cst.tile([128, NT * TCOLS], f32, tag="thr")
            nc.sync.dma_start(thr[:], thr_d.ap())
            iota = cst.tile([128, BLK], f32, tag="iota")
            nc.sync.dma_start(iota[:], iota_d.ap())

            tacc = accs.tile([128, NT, NG], f32, tag="tacc")
            lacc = accs.tile([128, NT, MAXC], f32, tag="lacc")
            hacc = accs.tile([128, NT, MAXC], f32, tag="hacc")
            s1r = accs.tile([128, NT], f32, tag="s1r")
            s3r = accs.tile([128, NT], f32, tag="s3r")

            for t in range(NT):
                st = struct[t]
                (lb0, lC, la) = st["lo"]
                (hb0, hC, ha) = st["hi"]
                for g in range(NG):
                    ps = psp.tile([128, GRP], f32, tag="ps")
                    for k in range(2):
                        for sb in range(GRP // BLK):
                            nc.tensor.matmul(
                                ps[:, sb * BLK:(sb + 1) * BLK],
                                xs[k][:, t * 128:(t + 1) * 128],
                                xg[k][g][:, sb * BLK:(sb + 1) * BLK],
                                start=(k == 0), stop=(k == 1),
                            )
                    et = expp.tile([128, GRP], f32, tag="et")
                    nc.scalar.activation(
                        et[:], ps[:], AF.Exp, scale=10.0,
                        accum_out=tacc[:, t, g:g + 1],
                    )
                    # boundary masked partial sums for candidate blocks in g
                    for side, b0, C, acc in (("lo", lb0, lC, lacc),
                                             ("hi", hb0, hC, hacc)):
                        for ci in range(C):
                            b = b0 + ci
                            if b * BLK // GRP != g:
                                continue
                            off = b * BLK - g * GRP
                            colbase = t * TCOLS + (0 if side == "lo" else MAXC)
                            junk = junkp.tile([128, BLK], f32, tag="junk")
                            nc.vector.scalar_tensor_tensor(
                                out=junk[:],
                                in0=iota[:],
                                scalar=thr[:, colbase + ci:colbase + ci + 1],
                                in1=et[:, off:off + BLK],
                                op0=OP.is_ge, op1=OP.mult,
                                accum_out=acc[:, t, ci:ci + 1],
                            )
                # per-tile assembly
                nc.vector.reduce_sum(s1r[:, t:t + 1], tacc[:, t, :],
                                     axis=mybir.AxisListType.X)
                u = tiny.tile([128, 1], f32, tag="u")
                nc.vector.reduce_sum(u[:], lacc[:, t, 0:lC],
                                     axis=mybir.AxisListType.X)
                w = tiny.tile([128, 1], f32, tag="w")
                nc.vector.reduce_sum(w[:], hacc[:, t, 0:hC],
                                     axis=mybir.AxisListType.X)
                if ha > la:
                    v = tiny.tile([128, 1], f32, tag="v")
                    nc.vector.reduce_sum(v[:], tacc[:, t, la:ha],
                                         axis=mybir.AxisListType.X)
                    uv = tiny.tile([128, 1], f32, tag="uv")
                    nc.vector.tensor_add(uv[:], u[:], v[:])
                else:
                    uv = u
                nc.vector.tensor_tensor(
                    out=s3r[:, t:t + 1], in0=uv[:], in1=w[:],
                    op=OP.subtract,
                )

            nc.sync.dma_start(s1_d.ap(), s1r[:])
            nc.sync.dma_start(s3_d.ap(), s3r[:])

    nc.compile()
    return nc


def _run_device(Xs: np.ndarray, lo: np.ndarray, hi: np.ndarray,
                trace: bool = False):
    """Xs: sorted embeddings [E, D] f32. lo/hi: band bounds per sorted row.

    Returns (S1, S3_band) per sorted row, and exec_time_ns if trace.
    """
    from concourse.bass_utils import run_bass_kernel_spmd

    Xr = _round_fp32r(Xs)
    hi1 = hi + 1

    # per-core rolled coordinates
    lo_l = np.empty((NCORES, SLAB), np.int64)
    hi1_l = np.empty((NCORES, SLAB), np.int64)
    rolls = []
    for c in range(NCORES):
        r = (c * SLAB - SHIFT) % E
        rolls.append(r)
        g0 = c * SLAB
        m_lo = (lo[g0:g0 + SLAB] - r) % E
        m_hi = (hi1[g0:g0 + SLAB] - 1 - r) % E
        assert (m_lo <= m_hi).all(), f"band wraps roll cut (core {c})"
        lo_l[c] = m_lo
        hi1_l[c] = m_hi + 1
    assert lo_l.min() >= 0 and hi1_l.max() <= E

    struct = _build_structure(lo_l, hi1_l)
    key = tuple((st["lo"], st["hi"]) for st in struct)
    if key not in _prog_cache:
        _prog_cache[key] = _build_program(struct)
    nc = _prog_cache[key]

    iota = np.broadcast_to(np.arange(BLK, dtype=np.float32), (128, BLK)).copy()
    in_maps = []
    for c in range(NCORES):
        r = rolls[c]
        Xroll = np.roll(Xr, -r, axis=0)              # [E, D]
        xa = np.ascontiguousarray(Xroll.T).reshape(2, 128, E)
        thr = np.zeros((128, NT * TCOLS), np.float32)
        for t in range(NT):
            rows = slice(t * 128, (t + 1) * 128)
            (lb0, lC, _la) = struct[t]["lo"]
            (hb0, hC, _ha) = struct[t]["hi"]
            for ci in range(lC):
                thr[:, t * TCOLS + ci] = (lo_l[c, rows] - (lb0 + ci) * BLK)
            for ci in range(hC):
                thr[:, t * TCOLS + MAXC + ci] = (hi1_l[c, rows] - (hb0 + ci) * BLK)
        in_maps.append({"xa": xa, "thr": thr, "iota": iota})

    _last_run["nc"] = nc
    _last_run["in_maps"] = in_maps
    res = run_bass_kernel_spmd(nc, in_maps, list(range(NCORES)))

    S1 = np.empty(E, np.float32)
    S3 = np.empty(E, np.float32)
    for c in range(NCORES):
        s1o = res.results[c]["s1o"]                  # [128, NT]
        s3o = res.results[c]["s3o"]
        for t in range(NT):
            g0 = c * SLAB + t * 128
            S1[g0:g0 + 128] = s1o[:, t]
            S3[g0:g0 + 128] = s3o[:, t]
    return S1, S3, res.exec_time_ns



def kernel(edge_embeddings, edge_index, structural_scores, _trace=False,
           _return_parts=False):
    X0 = np.asarray(edge_embeddings, dtype=np.float32)
    ei = np.asarray(edge_index)
    sc = np.asarray(structural_scores, dtype=np.float32)
    assert X0.shape == (E, D) and sc.shape == (E,)

    order = np.argsort(sc, kind="stable")
    s = sc[order]
    Xs = np.ascontiguousarray(X0[order])
    lo, hi = _band_bounds(s)

    # ---- host: positive_sim band part via float64 prefix sums ----
    Xd = Xs.astype(np.float64)
    P = np.zeros((E + 1, D), np.float64)
    np.cumsum(Xd, axis=0, out=P[1:])
    band_vec = P[hi + 1] - P[lo]
    pos_band = (Xd * band_vec).sum(axis=1) / 0.1

    # ---- host: sparse shared-endpoint corrections outside the band ----
    inv = np.empty(E, np.int64)
    inv[order] = np.arange(E)
    pos_corr = np.zeros(E, np.float64)
    s3_corr = np.zeros(E, np.float64)
    t01 = np.float32(0.1)
    for (a, b) in _shared_pairs(ei):
        i, j = int(inv[a]), int(inv[b])
        if np.float32(abs(np.float32(s[i] - s[j]))) < t01:
            continue  # already inside the score band
        simv = np.float64(np.float32(Xs[i] @ Xs[j]) / t01)
        ev = np.exp(simv)
        pos_corr[i] += simv
        pos_corr[j] += simv
        s3_corr[i] += ev
        s3_corr[j] += ev

    # ---- device: S1 = sum_j exp(sim), S3 = banded sum of exp(sim) ----
    S1, S3, exec_ns = _run_device(Xs, lo, hi, trace=_trace)

    pos = (pos_band + pos_corr).astype(np.float32)
    neg = (S1.astype(np.float64) - S3.astype(np.float64) - s3_corr
           ).astype(np.float32)
    with np.errstate(invalid="ignore", divide="ignore"):
        ratio = pos / (pos + neg + np.float32(1e-8))
        loss = -np.log(ratio)
    out = np.float32(np.mean(loss))
    if _return_parts:
        return out, dict(pos=pos, neg=neg, S1=S1, S3=S3, loss=loss,
                         order=order, exec_ns=exec_ns)
    return out
